# revision 32
# baseline (speedup 1.0000x reference)
"""BiLSTM + prototype-distance kernel for 8 trn2 NeuronCores.

v5 sharding: 8 cores = 2 directions x 4 SEQUENCE SEGMENTS, each core
carries the FULL batch of 32 rows. The LSTM forget gate contracts state
by ~0.5x/step, so a segment started from zero state converges to the
true state in ~32 steps; we run W=48 warmup steps (state error ~2e-7)
and discard their outputs. Per-core steps: 176 instead of 512.

v5 structure (per step, all batch-32):
- x@Wih for step t+1 runs during step t's activation chain, writing
  DIRECTLY into the (ping-ponged) G PSUM banks: two exact-bias matmuls
  (bf16 hi+lo) open the bank, 16 accumulating MMs add the embedding
  contribution. The recurrent h@Whh matmuls for step t+1 then
  accumulate into the same banks - no injection matmul, no xg ring.
- G split across PSUM banks A=[f,i,g] / B=[o]: sigmoid(f,i,g) on the
  chain, sigmoid(o) off it.
- Cell update v/u/c on DVE, tanh on ACT, h=o*tanh(c) bf16, hT2 via DVE
  32x32 stream transpose; Whh/protos host-permuted to the hT2 block
  convention (chunk k <-> hT2[:, 32k:32k+32]).
- ||h||^2 via 4 Gram matmuls on the proto PSUM tile (host reads diag).
- Embeds: GPSIMD indirect-gather (fp32) -> ACT bf16 cast -> DMA-xbar
  transposes (off the PE) -> embT.
"""

import sys
import numpy as np

sys.path.insert(0, "/opt/trn_rl_repo")

import concourse.bass as bass  # noqa: E402
import concourse.tile as tile  # noqa: E402
import concourse.mybir as mybir  # noqa: E402
from concourse import bacc  # noqa: E402
from concourse.bass_utils import run_bass_kernel_spmd  # noqa: E402

F32 = mybir.dt.float32
BF16 = mybir.dt.bfloat16
I32 = mybir.dt.int32

V, E, HD, P = 50000, 512, 1024, 128
H2 = HD // 2          # 512 per-direction hidden
B, T = 32, 512
NSEG = 4              # sequence segments per direction
WARM = 48             # warmup steps (state converges in ~32)
SEG = T // NSEG       # 128 real steps per segment
TLOC = SEG + WARM     # 176 steps per core
NG = TLOC // 4        # 44 granules
# gate order in G columns: f, i, g | o  (pytorch rows are i,f,g,o)
SRC = [1, 0, 2, 3]
OUTW = 160            # 128 proto cols + 32 gram cols per step


def _arrange_whh(w):
    """w: (2048, 512) -> A: (4, 128, 4*384), B: (4, 128, 4*128) in the
    hT2 convention: chunk k partition p <-> hidden 128*(p//32)+32k+(p%32)."""
    arrA = np.empty((4, 128, 4 * 384), np.float32)
    arrB = np.empty((4, 128, 4 * 128), np.float32)
    hi = np.arange(128)
    for k in range(4):
        hin = 128 * (hi // 32) + 32 * k + (hi % 32)
        for gam in range(4):
            scale = 2.0 if gam == 2 else 1.0
            blk = w[512 * SRC[gam]:512 * (SRC[gam] + 1), :][:, hin]  # (512,128)
            for c in range(4):
                sub = scale * blk[128 * c:128 * (c + 1), :].T        # (128,128)
                if gam < 3:
                    arrA[k, :, 384 * c + 128 * gam:384 * c + 128 * (gam + 1)] = sub
                else:
                    arrB[k, :, 128 * c:128 * (c + 1)] = sub
    return arrA, arrB


def _arrange_wih(w):
    """w: (2048, 512) -> A: (4, 128, 4*384), B: (4, 128, 4*128):
    contraction chunk k = embedding dims [128k, 128k+128) (plain order);
    gate column order f, i, g (A) | o (B), tanh gate doubled."""
    arrA = np.empty((4, 128, 4 * 384), np.float32)
    arrB = np.empty((4, 128, 4 * 128), np.float32)
    for k in range(4):
        for gam in range(4):
            scale = 2.0 if gam == 2 else 1.0
            blk = w[512 * SRC[gam]:512 * (SRC[gam] + 1),
                    128 * k:128 * (k + 1)]      # (512, 128)
            for c in range(4):
                sub = scale * blk[128 * c:128 * (c + 1), :].T
                if gam < 3:
                    arrA[k, :, 384 * c + 128 * gam:384 * c + 128 * (gam + 1)] = sub
                else:
                    arrB[k, :, 128 * c:128 * (c + 1)] = sub
    return arrA, arrB


def _arrange_b(b_total):
    """-> A (128, 384), B (128, 128) f32 in G layout."""
    bbA = np.zeros((128, 384), np.float32)
    bbB = np.zeros((128, 128), np.float32)
    for c in range(4):
        for gam in range(4):
            scale = 2.0 if gam == 2 else 1.0
            seg = scale * b_total[512 * SRC[gam] + 128 * c:
                                  512 * SRC[gam] + 128 * (c + 1)]
            if gam < 3:
                bbA[32 * c:32 * (c + 1), 128 * gam:128 * (gam + 1)] = seg[None, :]
            else:
                bbB[32 * c:32 * (c + 1), :] = seg[None, :]
    return bbA, bbB


def _arrange_pt(prot_half):
    """prot_half: (128, 512) -> (4, 128, 128) in the hT2 convention."""
    hi = np.arange(128)
    cc, jp = hi // 32, hi % 32
    arr = np.empty((4, 128, 128), np.float32)
    for k in range(4):
        hin = 128 * cc + 32 * k + jp
        arr[k] = prot_half[:, hin].T
    return arr


def _arrange_idx(ids_seg):
    """ids_seg: (32, TLOC) -> (128, NG) int32: [32*tt + b, g] = ids[b, 4g+tt]."""
    idx = np.zeros((128, NG), np.int32)
    for g in range(NG):
        for tt in range(4):
            idx[32 * tt:32 * (tt + 1), g] = ids_seg[:, 4 * g + tt]
    return idx


def build_program(n_gran=NG):
    nc = bacc.Bacc("TRN2", target_bir_lowering=False, debug=False)

    emb = nc.dram_tensor("emb", [V, E], F32, kind="ExternalInput").ap()
    idx_d = nc.dram_tensor("idx", [128, n_gran], I32, kind="ExternalInput").ap()
    wiA_d = nc.dram_tensor("wiA", [4, 128, 4 * 384], BF16, kind="ExternalInput").ap()
    wiB_d = nc.dram_tensor("wiB", [4, 128, 4 * 128], BF16, kind="ExternalInput").ap()
    whA_d = nc.dram_tensor("whA", [4, 128, 4 * 384], BF16, kind="ExternalInput").ap()
    whB_d = nc.dram_tensor("whB", [4, 128, 4 * 128], BF16, kind="ExternalInput").ap()
    bbA_d = nc.dram_tensor("bbA", [2, 128, 384], BF16, kind="ExternalInput").ap()
    bbB_d = nc.dram_tensor("bbB", [2, 128, 128], BF16, kind="ExternalInput").ap()
    pt_d = nc.dram_tensor("pt", [4, 128, 128], BF16, kind="ExternalInput").ap()

    Tloc = 4 * n_gran
    xp_d = nc.dram_tensor("xp", [64, Tloc * OUTW], F32, kind="ExternalOutput").ap()

    with tile.TileContext(nc) as tc:
        _body(tc, n_gran, emb, idx_d, wiA_d, wiB_d, whA_d, whB_d, bbA_d,
              bbB_d, pt_d, xp_d)

    nc.compile()
    return nc


def _body(tc, n_gran, emb, idx_d, wiA_d, wiB_d, whA_d, whB_d, bbA_d, bbB_d,
          pt_d, xp_d):
    nc = tc.nc
    from contextlib import ExitStack
    ctx = ExitStack()
    const = ctx.enter_context(tc.tile_pool(name="const", bufs=1))
    state = ctx.enter_context(tc.tile_pool(name="state", bufs=1))
    work = ctx.enter_context(tc.tile_pool(name="work", bufs=2))
    psum_a = ctx.enter_context(tc.tile_pool(name="psa", bufs=2, space="PSUM"))
    psum_b = ctx.enter_context(tc.tile_pool(name="psb", bufs=2, space="PSUM"))
    psum_p = ctx.enter_context(tc.tile_pool(name="psp", bufs=2, space="PSUM"))

    # ---- resident tensors -------------------------------------------------
    wiA = const.tile([128, 4 * 4 * 384], BF16)
    wiB = const.tile([128, 4 * 4 * 128], BF16)
    whA = const.tile([128, 4 * 4 * 384], BF16)
    whB = const.tile([128, 4 * 4 * 128], BF16)
    bbA = const.tile([128, 2 * 384], BF16)      # hi | lo
    bbB = const.tile([128, 2 * 128], BF16)
    pt = const.tile([128, 4 * 128], BF16)
    idx = const.tile([128, n_gran], I32)

    for k in range(4):
        nc.sync.dma_start(wiA[:, 1536 * k:1536 * (k + 1)], wiA_d[k])
        nc.sync.dma_start(wiB[:, 512 * k:512 * (k + 1)], wiB_d[k])
        nc.sync.dma_start(whA[:, 1536 * k:1536 * (k + 1)], whA_d[k])
        nc.sync.dma_start(whB[:, 512 * k:512 * (k + 1)], whB_d[k])
        nc.sync.dma_start(pt[:, 128 * k:128 * (k + 1)], pt_d[k])
    for h in range(2):
        nc.sync.dma_start(bbA[:, 384 * h:384 * (h + 1)], bbA_d[h])
        nc.sync.dma_start(bbB[:, 128 * h:128 * (h + 1)], bbB_d[h])
    nc.sync.dma_start(idx[:], idx_d[:])

    # state
    c_st = state.tile([128, 128], F32)
    hT2 = state.tile([128, 2 * 128], BF16)          # ping-pong on t%2
    emb_ring = state.tile([128, 4 * 512], F32)      # slot = g%4 (gather dst)
    embb_ring = state.tile([128, 4 * 512], BF16)    # slot = g%4 (bf16 cast)
    embT = state.tile([128, 4 * 512], BF16)         # slot = g%4; [4k x (tt,b)]
    out_ring = state.tile([96, 16 * OUTW], F32)     # rows 32:96; host adds

    nc.gpsimd.memset(c_st[:], 0.0)
    nc.gpsimd.memset(hT2[:], 0.0)
    nc.gpsimd.memset(emb_ring[:], 0.0)
    nc.gpsimd.memset(embb_ring[:], 0.0)
    nc.gpsimd.memset(embT[:], 0.0)
    nc.gpsimd.memset(out_ring[:], 0.0)

    def gather(g):
        s = 512 * (g % 4)
        nc.gpsimd.indirect_dma_start(
            out=emb_ring[:, s:s + 512],
            out_offset=None,
            in_=emb[:],
            in_offset=bass.IndirectOffsetOnAxis(ap=idx[:, g:g + 1], axis=0),
        )

    def embt_granule(g):
        """bf16 cast (ACT) + 4 DMA-xbar transposes -> embT slot g%4."""
        s, s2 = 512 * (g % 4), 512 * (g % 4)
        nc.scalar.copy(embb_ring[:, s2:s2 + 512], emb_ring[:, s:s + 512])
        for k in range(4):
            nc.sync.dma_start_transpose(
                embT[:, s2 + 128 * k:s2 + 128 * (k + 1)],
                embb_ring[:, s2 + 128 * k:s2 + 128 * (k + 1)])

    g_tiles = {}

    def phase1_step(t):
        """xg for step t -> fresh GA/GB psum tiles (bias + x@Wih)."""
        g, tt = t // 4, t % 4
        s2 = 512 * (g % 4)
        GA = psum_a.tile([128, 384], F32, tag="ga")
        GB = psum_b.tile([128, 128], F32, tag="gb")
        g_tiles[t] = (GA, GB)
        # bias inject (bf16-rounded; residual ~6e-4 on outputs)
        nc.tensor.matmul(GA[:], lhsT=identb[:], rhs=bbA[:, 0:384],
                         start=True, stop=False)
        nc.tensor.matmul(GB[:], lhsT=identb[:], rhs=bbB[:, 0:128],
                         start=True, stop=False)
        for k in range(4):
            et = embT[:, s2 + 128 * k + 32 * tt:s2 + 128 * k + 32 * (tt + 1)]
            for c in range(4):
                nc.tensor.matmul(
                    GA[32 * c:32 * c + 32, :],
                    lhsT=et,
                    rhs=wiA[:, 1536 * k + 384 * c:1536 * k + 384 * (c + 1)],
                    start=False, stop=False,
                    tile_position=(0, 32 * c))
            for c in range(4):
                nc.tensor.matmul(
                    GB[32 * c:32 * c + 32, :],
                    lhsT=et,
                    rhs=wiB[:, 512 * k + 128 * c:512 * k + 128 * (c + 1)],
                    start=False, stop=False,
                    tile_position=(0, 32 * c))

    def step_mms(t):
        """h@Whh accumulating into the phase1-opened banks."""
        GA, GB = g_tiles.pop(t)
        cur = hT2[:, 128 * (t % 2):128 * (t % 2) + 128]
        for k in range(4):
            for c in range(4):
                nc.tensor.matmul(
                    GA[32 * c:32 * c + 32, :],
                    lhsT=cur[:, 32 * k:32 * k + 32],
                    rhs=whA[:, 1536 * k + 384 * c:1536 * k + 384 * (c + 1)],
                    start=False, stop=(k == 3),
                    tile_position=(0, 32 * c))
        for k in range(4):
            for c in range(4):
                nc.tensor.matmul(
                    GB[32 * c:32 * c + 32, :],
                    lhsT=cur[:, 32 * k:32 * k + 32],
                    rhs=whB[:, 512 * k + 128 * c:512 * k + 128 * (c + 1)],
                    start=False, stop=(k == 3),
                    tile_position=(0, 32 * c))
        return GA, GB

    def chain(t, GA, GB):
        nxt = hT2[:, 128 * ((t + 1) % 2):128 * ((t + 1) % 2) + 128]
        gh = work.tile([128, 384], F32, tag="gh")
        gho = work.tile([128, 128], F32, tag="gho")
        nc.scalar.activation(gh[:], GA[:], mybir.ActivationFunctionType.Sigmoid)
        nc.scalar.activation(gho[:], GB[:], mybir.ActivationFunctionType.Sigmoid)
        u = work.tile([128, 128], F32, tag="u")
        v = work.tile([128, 128], F32, tag="v")
        nc.vector.tensor_tensor(out=v[:], in0=gh[:, 0:128], in1=c_st[:],
                                op=mybir.AluOpType.mult)
        nc.vector.scalar_tensor_tensor(
            out=u[:], in0=gh[:, 256:384], scalar=0.5, in1=gh[:, 128:256],
            op0=mybir.AluOpType.subtract, op1=mybir.AluOpType.mult)
        nc.vector.scalar_tensor_tensor(
            out=c_st[:], in0=u[:], scalar=2.0, in1=v[:],
            op0=mybir.AluOpType.mult, op1=mybir.AluOpType.add)
        tc_t = work.tile([128, 128], F32, tag="tc")
        nc.scalar.activation(tc_t[:], c_st[:], mybir.ActivationFunctionType.Tanh)
        h_sb = work.tile([128, 128], BF16, tag="h")
        nc.vector.tensor_tensor(out=h_sb[:], in0=gho[:], in1=tc_t[:],
                                op=mybir.AluOpType.mult)
        nc.vector.transpose(nxt, h_sb[:])

    def proto_for_state(buf):
        """proto+gram as two half-sums on strips 1 and 2 (keeps strip 0
        free for the gate matmuls); emit_out adds the halves."""
        cur = hT2[:, 128 * buf:128 * buf + 128]
        pp = psum_p.tile([128, OUTW], F32)
        for h in range(2):
            sl = slice(32 + 32 * h, 64 + 32 * h)
            for kk in range(2):
                k = 2 * h + kk
                nc.tensor.matmul(pp[sl, 0:128],
                                 lhsT=cur[:, 32 * k:32 * k + 32],
                                 rhs=pt[:, 128 * k:128 * (k + 1)],
                                 start=(kk == 0), stop=False,
                                 tile_position=(0, 32 + 32 * h))
                nc.tensor.matmul(pp[sl, 128:160],
                                 lhsT=cur[:, 32 * k:32 * k + 32],
                                 rhs=cur[:, 32 * k:32 * k + 32],
                                 start=False, stop=(kk == 1),
                                 tile_position=(0, 32 + 32 * h))
        return pp

    def emit_out(tprev, pp):
        col = OUTW * (tprev % 16)
        nc.vector.tensor_copy(out_ring[32:64, col:col + OUTW], pp[32:64, :])
        nc.vector.tensor_copy(out_ring[64:96, col:col + OUTW], pp[64:96, :])
        if tprev % 16 == 15:
            blk = (tprev - 15) * OUTW
            nc.sync.dma_start(xp_d[:, blk:blk + 16 * OUTW], out_ring[32:96, :])

    # identity for the bias matmuls (declared late so make_identity's
    # gpsimd ops sit after the big memsets)
    identb = const.tile([128, 128], BF16)
    from concourse.masks import make_identity
    make_identity(nc, identb[:])

    # ---- main loop --------------------------------------------------------
    for g in range(3):
        gather(g)
    for g in range(2):
        embt_granule(g)
    phase1_step(0)
    for g in range(n_gran):
        if g + 3 < n_gran:
            gather(g + 3)
        for tt in range(4):
            t = 4 * g + tt
            GA, GB = step_mms(t)
            pp = proto_for_state(t % 2) if t > 0 else None
            if t + 1 < 4 * n_gran:
                phase1_step(t + 1)
            chain(t, GA, GB)
            if pp is not None:
                emit_out(t - 1, pp)
        if g + 2 < n_gran:
            embt_granule(g + 2)
    pp = proto_for_state((4 * n_gran) % 2)
    emit_out(4 * n_gran - 1, pp)
    ctx.close()


def _segment_ids(ids_dir):
    """ids_dir: (32, 512) direction-ordered ids -> per-segment (32, TLOC)."""
    segs = []
    for s in range(NSEG):
        lo = SEG * s - (WARM if s > 0 else 0)
        hi = lo + TLOC
        if hi <= T:
            seg = ids_dir[:, lo:hi]
        else:
            pad = np.repeat(ids_dir[:, -1:], hi - T, axis=1)
            seg = np.concatenate([ids_dir[:, lo:], pad], axis=1)
        segs.append(np.ascontiguousarray(seg))
    return segs


def _prep_inputs(input_ids, embed_table, w_ih_f, w_hh_f, b_ih_f, b_hh_f,
                 w_ih_b, w_hh_b, b_ih_b, b_hh_b, prototypes, n_gran=NG):
    import ml_dtypes
    bf16 = ml_dtypes.bfloat16
    ids = np.asarray(input_ids).astype(np.int32)
    emb = np.ascontiguousarray(np.asarray(embed_table, np.float32))
    prot = np.asarray(prototypes, np.float32)
    per_dir = {}
    for d, (wi, wh, bi, bh) in enumerate([
            (w_ih_f, w_hh_f, b_ih_f, b_hh_f),
            (w_ih_b, w_hh_b, b_ih_b, b_hh_b)]):
        wiA, wiB = _arrange_wih(np.asarray(wi, np.float32))
        whA, whB = _arrange_whh(np.asarray(wh, np.float32))
        bA, bB = _arrange_b(np.asarray(bi, np.float32)
                            + np.asarray(bh, np.float32))
        bA_hi = bA.astype(bf16)
        bA_lo = (bA - bA_hi.astype(np.float32)).astype(bf16)
        bB_hi = bB.astype(bf16)
        bB_lo = (bB - bB_hi.astype(np.float32)).astype(bf16)
        per_dir[d] = dict(
            wiA=np.ascontiguousarray(wiA).astype(bf16),
            wiB=np.ascontiguousarray(wiB).astype(bf16),
            whA=np.ascontiguousarray(whA).astype(bf16),
            whB=np.ascontiguousarray(whB).astype(bf16),
            bbA=np.ascontiguousarray(np.stack([bA_hi, bA_lo])),
            bbB=np.ascontiguousarray(np.stack([bB_hi, bB_lo])),
            pt=np.ascontiguousarray(
                _arrange_pt(prot[:, 512 * d:512 * (d + 1)])).astype(bf16),
        )
    in_maps = []
    for core in range(8):
        d, s = core // 4, core % 4
        ids_dir = ids if d == 0 else ids[:, ::-1]
        seg = _segment_ids(ids_dir)[s]
        pd = per_dir[d]
        in_maps.append(dict(
            emb=emb, idx=_arrange_idx(seg),
            wiA=pd["wiA"], wiB=pd["wiB"], whA=pd["whA"], whB=pd["whB"],
            bbA=pd["bbA"], bbB=pd["bbB"], pt=pd["pt"],
        ))
    return in_maps


def _combine(results, prototypes, n_gran=NG):
    p2 = (np.asarray(prototypes, np.float32) ** 2).sum(-1)  # (128,)
    out = np.zeros((32, T, 128), np.float32)
    bidx = np.arange(32)
    for core in range(8):
        d, s = core // 4, core % 4
        raw = results[core]["xp"].reshape(2, 32, TLOC, OUTW)
        blocks = raw[0] + raw[1]
        xp = blocks[:, :, 0:128]                       # (32, TLOC, 128)
        x2 = blocks[bidx, :, 128 + bidx]               # (32, TLOC)
        if s == 0:
            lo_l, hi_l, lo_t = 0, min(TLOC, SEG), 0
        else:
            lo_l = WARM
            lo_t = SEG * s
            hi_l = min(TLOC, WARM + min(SEG, T - lo_t))
        xp_r = xp[:, lo_l:hi_l]
        x2_r = x2[:, lo_l:hi_l]
        tdir = np.arange(lo_t, lo_t + hi_l - lo_l)
        tglob = tdir if d == 0 else T - 1 - tdir
        out[:, tglob, :] += 2.0 * xp_r - x2_r[:, :, None]
    out -= p2[None, None, :]
    return out


_NC_CACHE = {}


def kernel(input_ids, embed_table, w_ih_f, w_hh_f, b_ih_f, b_hh_f,
           w_ih_b, w_hh_b, b_ih_b, b_hh_b, prototypes):
    n_gran = NG
    if n_gran not in _NC_CACHE:
        _NC_CACHE[n_gran] = build_program(n_gran)
    nc = _NC_CACHE[n_gran]
    in_maps = _prep_inputs(input_ids, embed_table, w_ih_f, w_hh_f, b_ih_f,
                           b_hh_f, w_ih_b, w_hh_b, b_ih_b, b_hh_b, prototypes,
                           n_gran)
    res = run_bass_kernel_spmd(nc, in_maps, list(range(8)))
    return _combine(res.results, prototypes, n_gran)


if __name__ == "__main__":
    import time
    t0 = time.time()
    ng = int(sys.argv[1]) if len(sys.argv) > 1 else NG
    nc = build_program(ng)
    print(f"built n_gran={ng} in {time.time()-t0:.1f}s")


# revision 33
# speedup vs baseline: 1.1738x; 1.1738x over previous
"""BiLSTM + prototype-distance kernel for 8 trn2 NeuronCores.

v5 sharding: 8 cores = 2 directions x 4 SEQUENCE SEGMENTS, each core
carries the FULL batch of 32 rows. The LSTM forget gate contracts state
by ~0.5x/step, so a segment started from zero state converges to the
true state in ~32 steps; we run W=48 warmup steps (state error ~2e-7)
and discard their outputs. Per-core steps: 176 instead of 512.

v5 structure (per step, all batch-32):
- x@Wih for step t+1 runs during step t's activation chain, writing
  DIRECTLY into the (ping-ponged) G PSUM banks: two exact-bias matmuls
  (bf16 hi+lo) open the bank, 16 accumulating MMs add the embedding
  contribution. The recurrent h@Whh matmuls for step t+1 then
  accumulate into the same banks - no injection matmul, no xg ring.
- G split across PSUM banks A=[f,i,g] / B=[o]: sigmoid(f,i,g) on the
  chain, sigmoid(o) off it.
- Cell update v/u/c on DVE, tanh on ACT, h=o*tanh(c) bf16, hT2 via DVE
  32x32 stream transpose; Whh/protos host-permuted to the hT2 block
  convention (chunk k <-> hT2[:, 32k:32k+32]).
- ||h||^2 via 4 Gram matmuls on the proto PSUM tile (host reads diag).
- Embeds: GPSIMD indirect-gather (fp32) -> ACT bf16 cast -> DMA-xbar
  transposes (off the PE) -> embT.
"""

import sys
import numpy as np

sys.path.insert(0, "/opt/trn_rl_repo")

import concourse.bass as bass  # noqa: E402
import concourse.tile as tile  # noqa: E402
import concourse.mybir as mybir  # noqa: E402
from concourse import bacc  # noqa: E402
from concourse.bass_utils import run_bass_kernel_spmd  # noqa: E402

F32 = mybir.dt.float32
BF16 = mybir.dt.bfloat16
I32 = mybir.dt.int32

V, E, HD, P = 50000, 512, 1024, 128
H2 = HD // 2          # 512 per-direction hidden
B, T = 32, 512
NSEG = 4              # sequence segments per direction
WARM = 48             # warmup steps (state converges in ~32)
SEG = T // NSEG       # 128 real steps per segment
TLOC = SEG + WARM     # 176 steps per core
NG = TLOC // 4        # 44 granules
# gate order in G columns: f, i, g | o  (pytorch rows are i,f,g,o)
SRC = [1, 0, 2, 3]
OUTW = 160            # 128 proto cols + 32 gram cols per step


def _arrange_whh(w):
    """w: (2048, 512) -> A: (4, 128, 4*384), B: (4, 128, 4*128) in the
    hT2 convention: chunk k partition p <-> hidden 128*(p//32)+32k+(p%32)."""
    arrA = np.empty((4, 128, 4 * 384), np.float32)
    arrB = np.empty((4, 128, 4 * 128), np.float32)
    hi = np.arange(128)
    for k in range(4):
        hin = 128 * (hi // 32) + 32 * k + (hi % 32)
        for gam in range(4):
            scale = 2.0 if gam == 2 else 1.0
            blk = w[512 * SRC[gam]:512 * (SRC[gam] + 1), :][:, hin]  # (512,128)
            for c in range(4):
                sub = scale * blk[128 * c:128 * (c + 1), :].T        # (128,128)
                if gam < 3:
                    arrA[k, :, 384 * c + 128 * gam:384 * c + 128 * (gam + 1)] = sub
                else:
                    arrB[k, :, 128 * c:128 * (c + 1)] = sub
    return arrA, arrB


def _arrange_wih(w):
    """w: (2048, 512) -> A: (4, 128, 4*384), B: (4, 128, 4*128):
    contraction chunk k = embedding dims [128k, 128k+128) (plain order);
    gate column order f, i, g (A) | o (B), tanh gate doubled."""
    arrA = np.empty((4, 128, 4 * 384), np.float32)
    arrB = np.empty((4, 128, 4 * 128), np.float32)
    for k in range(4):
        for gam in range(4):
            scale = 2.0 if gam == 2 else 1.0
            blk = w[512 * SRC[gam]:512 * (SRC[gam] + 1),
                    128 * k:128 * (k + 1)]      # (512, 128)
            for c in range(4):
                sub = scale * blk[128 * c:128 * (c + 1), :].T
                if gam < 3:
                    arrA[k, :, 384 * c + 128 * gam:384 * c + 128 * (gam + 1)] = sub
                else:
                    arrB[k, :, 128 * c:128 * (c + 1)] = sub
    return arrA, arrB


def _arrange_b(b_total):
    """-> A (128, 384), B (128, 128) f32 in G layout."""
    bbA = np.zeros((128, 384), np.float32)
    bbB = np.zeros((128, 128), np.float32)
    for c in range(4):
        for gam in range(4):
            scale = 2.0 if gam == 2 else 1.0
            seg = scale * b_total[512 * SRC[gam] + 128 * c:
                                  512 * SRC[gam] + 128 * (c + 1)]
            if gam < 3:
                bbA[32 * c:32 * (c + 1), 128 * gam:128 * (gam + 1)] = seg[None, :]
            else:
                bbB[32 * c:32 * (c + 1), :] = seg[None, :]
    return bbA, bbB


def _arrange_pt(prot_half):
    """prot_half: (128, 512) -> (4, 128, 128) in the hT2 convention."""
    hi = np.arange(128)
    cc, jp = hi // 32, hi % 32
    arr = np.empty((4, 128, 128), np.float32)
    for k in range(4):
        hin = 128 * cc + 32 * k + jp
        arr[k] = prot_half[:, hin].T
    return arr


def _arrange_idx(ids_seg):
    """ids_seg: (32, TLOC) -> (128, NG) int32: [32*tt + b, g] = ids[b, 4g+tt]."""
    idx = np.zeros((128, NG), np.int32)
    for g in range(NG):
        for tt in range(4):
            idx[32 * tt:32 * (tt + 1), g] = ids_seg[:, 4 * g + tt]
    return idx


def build_program(n_gran=NG):
    nc = bacc.Bacc("TRN2", target_bir_lowering=False, debug=False)

    emb = nc.dram_tensor("emb", [V, E], F32, kind="ExternalInput").ap()
    idx_d = nc.dram_tensor("idx", [128, n_gran], I32, kind="ExternalInput").ap()
    wiA_d = nc.dram_tensor("wiA", [4, 128, 4 * 384], BF16, kind="ExternalInput").ap()
    wiB_d = nc.dram_tensor("wiB", [4, 128, 4 * 128], BF16, kind="ExternalInput").ap()
    whA_d = nc.dram_tensor("whA", [4, 128, 4 * 384], BF16, kind="ExternalInput").ap()
    whB_d = nc.dram_tensor("whB", [4, 128, 4 * 128], BF16, kind="ExternalInput").ap()
    bbA_d = nc.dram_tensor("bbA", [2, 128, 384], BF16, kind="ExternalInput").ap()
    bbB_d = nc.dram_tensor("bbB", [2, 128, 128], BF16, kind="ExternalInput").ap()
    pt_d = nc.dram_tensor("pt", [4, 128, 128], BF16, kind="ExternalInput").ap()

    Tloc = 4 * n_gran
    xp_d = nc.dram_tensor("xp", [64, Tloc * OUTW], F32, kind="ExternalOutput").ap()

    with tile.TileContext(nc) as tc:
        _body(tc, n_gran, emb, idx_d, wiA_d, wiB_d, whA_d, whB_d, bbA_d,
              bbB_d, pt_d, xp_d)

    nc.compile()
    return nc


def _body(tc, n_gran, emb, idx_d, wiA_d, wiB_d, whA_d, whB_d, bbA_d, bbB_d,
          pt_d, xp_d):
    nc = tc.nc
    from contextlib import ExitStack
    ctx = ExitStack()
    const = ctx.enter_context(tc.tile_pool(name="const", bufs=1))
    state = ctx.enter_context(tc.tile_pool(name="state", bufs=1))
    work = ctx.enter_context(tc.tile_pool(name="work", bufs=2))
    psum_a = ctx.enter_context(tc.tile_pool(name="psa", bufs=2, space="PSUM"))
    psum_b = ctx.enter_context(tc.tile_pool(name="psb", bufs=2, space="PSUM"))
    psum_p = ctx.enter_context(tc.tile_pool(name="psp", bufs=2, space="PSUM"))
    psum_t = ctx.enter_context(tc.tile_pool(name="pst", bufs=1, space="PSUM"))

    # ---- resident tensors -------------------------------------------------
    wiA = const.tile([128, 4 * 4 * 384], BF16)
    wiB = const.tile([128, 4 * 4 * 128], BF16)
    whA = const.tile([128, 4 * 4 * 384], BF16)
    whB = const.tile([128, 4 * 4 * 128], BF16)
    bbA = const.tile([128, 2 * 384], BF16)      # hi | lo
    bbB = const.tile([128, 2 * 128], BF16)
    pt = const.tile([128, 4 * 128], BF16)
    idx = const.tile([128, n_gran], I32)

    for k in range(4):
        nc.sync.dma_start(wiA[:, 1536 * k:1536 * (k + 1)], wiA_d[k])
        nc.sync.dma_start(wiB[:, 512 * k:512 * (k + 1)], wiB_d[k])
        nc.sync.dma_start(whA[:, 1536 * k:1536 * (k + 1)], whA_d[k])
        nc.sync.dma_start(whB[:, 512 * k:512 * (k + 1)], whB_d[k])
        nc.sync.dma_start(pt[:, 128 * k:128 * (k + 1)], pt_d[k])
    for h in range(2):
        nc.sync.dma_start(bbA[:, 384 * h:384 * (h + 1)], bbA_d[h])
        nc.sync.dma_start(bbB[:, 128 * h:128 * (h + 1)], bbB_d[h])
    nc.sync.dma_start(idx[:], idx_d[:])

    # state
    c_st = state.tile([128, 128], F32)
    hT2 = state.tile([128, 2 * 128], BF16)          # ping-pong on t%2
    emb_ring = state.tile([128, 4 * 512], F32)      # slot = g%4 (gather dst)
    embb_ring = state.tile([128, 4 * 512], BF16)    # slot = g%4 (bf16 cast)
    embT = state.tile([128, 4 * 512], BF16)         # slot = g%4; [4k x (tt,b)]
    out_ring = state.tile([96, 16 * OUTW], F32)     # rows 32:96; host adds

    nc.gpsimd.memset(c_st[:], 0.0)
    nc.gpsimd.memset(hT2[:], 0.0)
    nc.gpsimd.memset(emb_ring[:], 0.0)
    nc.gpsimd.memset(embb_ring[:], 0.0)
    nc.gpsimd.memset(embT[:], 0.0)
    nc.gpsimd.memset(out_ring[:], 0.0)

    def gather(g):
        s = 512 * (g % 4)
        nc.gpsimd.indirect_dma_start(
            out=emb_ring[:, s:s + 512],
            out_offset=None,
            in_=emb[:],
            in_offset=bass.IndirectOffsetOnAxis(ap=idx[:, g:g + 1], axis=0),
        )

    def embt_granule(g):
        """bf16 cast (ACT) + 4 PE transposes + ACT copies -> embT g%4."""
        s, s2 = 512 * (g % 4), 512 * (g % 4)
        nc.scalar.copy(embb_ring[:, s2:s2 + 512], emb_ring[:, s:s + 512])
        for k in range(4):
            tp = psum_t.tile([128, 128], BF16, tag="tp")
            nc.tensor.matmul(
                tp[:], lhsT=embb_ring[:, s2 + 128 * k:s2 + 128 * (k + 1)],
                rhs=identb[:], is_transpose=True, start=True, stop=True)
            nc.scalar.copy(embT[:, s2 + 128 * k:s2 + 128 * (k + 1)], tp[:])

    g_tiles = {}

    def phase1_step(t):
        """xg for step t -> fresh GA/GB psum tiles (bias + x@Wih)."""
        g, tt = t // 4, t % 4
        s2 = 512 * (g % 4)
        GA = psum_a.tile([128, 384], F32, tag="ga")
        GB = psum_b.tile([128, 128], F32, tag="gb")
        g_tiles[t] = (GA, GB)
        # bias inject (bf16-rounded; residual ~6e-4 on outputs)
        nc.tensor.matmul(GA[:], lhsT=identb[:], rhs=bbA[:, 0:384],
                         start=True, stop=False)
        nc.tensor.matmul(GB[:], lhsT=identb[:], rhs=bbB[:, 0:128],
                         start=True, stop=False)
        for k in range(4):
            et = embT[:, s2 + 128 * k + 32 * tt:s2 + 128 * k + 32 * (tt + 1)]
            for c in range(4):
                nc.tensor.matmul(
                    GA[32 * c:32 * c + 32, :],
                    lhsT=et,
                    rhs=wiA[:, 1536 * k + 384 * c:1536 * k + 384 * (c + 1)],
                    start=False, stop=False,
                    tile_position=(0, 32 * c))
            for c in range(4):
                nc.tensor.matmul(
                    GB[32 * c:32 * c + 32, :],
                    lhsT=et,
                    rhs=wiB[:, 512 * k + 128 * c:512 * k + 128 * (c + 1)],
                    start=False, stop=False,
                    tile_position=(0, 32 * c))

    def step_mms(t):
        """h@Whh accumulating into the phase1-opened banks."""
        GA, GB = g_tiles.pop(t)
        cur = hT2[:, 128 * (t % 2):128 * (t % 2) + 128]
        for k in range(4):
            for c in range(4):
                nc.tensor.matmul(
                    GA[32 * c:32 * c + 32, :],
                    lhsT=cur[:, 32 * k:32 * k + 32],
                    rhs=whA[:, 1536 * k + 384 * c:1536 * k + 384 * (c + 1)],
                    start=False, stop=(k == 3),
                    tile_position=(0, 32 * c))
        for k in range(4):
            for c in range(4):
                nc.tensor.matmul(
                    GB[32 * c:32 * c + 32, :],
                    lhsT=cur[:, 32 * k:32 * k + 32],
                    rhs=whB[:, 512 * k + 128 * c:512 * k + 128 * (c + 1)],
                    start=False, stop=(k == 3),
                    tile_position=(0, 32 * c))
        return GA, GB

    def chain(t, GA, GB):
        nxt = hT2[:, 128 * ((t + 1) % 2):128 * ((t + 1) % 2) + 128]
        gh = work.tile([128, 384], F32, tag="gh")
        gho = work.tile([128, 128], F32, tag="gho")
        nc.scalar.activation(gh[:], GA[:], mybir.ActivationFunctionType.Sigmoid)
        nc.scalar.activation(gho[:], GB[:], mybir.ActivationFunctionType.Sigmoid)
        u = work.tile([128, 128], F32, tag="u")
        v = work.tile([128, 128], F32, tag="v")
        nc.vector.tensor_tensor(out=v[:], in0=gh[:, 0:128], in1=c_st[:],
                                op=mybir.AluOpType.mult)
        nc.vector.scalar_tensor_tensor(
            out=u[:], in0=gh[:, 256:384], scalar=0.5, in1=gh[:, 128:256],
            op0=mybir.AluOpType.subtract, op1=mybir.AluOpType.mult)
        nc.vector.scalar_tensor_tensor(
            out=c_st[:], in0=u[:], scalar=2.0, in1=v[:],
            op0=mybir.AluOpType.mult, op1=mybir.AluOpType.add)
        tc_t = work.tile([128, 128], F32, tag="tc")
        nc.scalar.activation(tc_t[:], c_st[:], mybir.ActivationFunctionType.Tanh)
        h_sb = work.tile([128, 128], BF16, tag="h")
        nc.vector.tensor_tensor(out=h_sb[:], in0=gho[:], in1=tc_t[:],
                                op=mybir.AluOpType.mult)
        nc.vector.transpose(nxt, h_sb[:])

    def proto_for_state(buf):
        """proto+gram as two half-sums on strips 1 and 2 (keeps strip 0
        free for the gate matmuls); emit_out adds the halves."""
        cur = hT2[:, 128 * buf:128 * buf + 128]
        pp = psum_p.tile([128, OUTW], F32)
        for h in range(2):
            sl = slice(32 + 32 * h, 64 + 32 * h)
            for kk in range(2):
                k = 2 * h + kk
                nc.tensor.matmul(pp[sl, 0:128],
                                 lhsT=cur[:, 32 * k:32 * k + 32],
                                 rhs=pt[:, 128 * k:128 * (k + 1)],
                                 start=(kk == 0), stop=False,
                                 tile_position=(0, 32 + 32 * h))
                nc.tensor.matmul(pp[sl, 128:160],
                                 lhsT=cur[:, 32 * k:32 * k + 32],
                                 rhs=cur[:, 32 * k:32 * k + 32],
                                 start=False, stop=(kk == 1),
                                 tile_position=(0, 32 + 32 * h))
        return pp

    def emit_out(tprev, pp):
        col = OUTW * (tprev % 16)
        nc.vector.tensor_copy(out_ring[32:64, col:col + OUTW], pp[32:64, :])
        nc.vector.tensor_copy(out_ring[64:96, col:col + OUTW], pp[64:96, :])
        if tprev % 16 == 15:
            blk = (tprev - 15) * OUTW
            nc.sync.dma_start(xp_d[:, blk:blk + 16 * OUTW], out_ring[32:96, :])

    # identity for the bias matmuls (declared late so make_identity's
    # gpsimd ops sit after the big memsets)
    identb = const.tile([128, 128], BF16)
    from concourse.masks import make_identity
    make_identity(nc, identb[:])

    # ---- main loop --------------------------------------------------------
    for g in range(3):
        gather(g)
    for g in range(2):
        embt_granule(g)
    phase1_step(0)
    for g in range(n_gran):
        if g + 3 < n_gran:
            gather(g + 3)
        for tt in range(4):
            t = 4 * g + tt
            GA, GB = step_mms(t)
            pp = proto_for_state(t % 2) if t > 0 else None
            if t + 1 < 4 * n_gran:
                phase1_step(t + 1)
            chain(t, GA, GB)
            if pp is not None:
                emit_out(t - 1, pp)
        if g + 2 < n_gran:
            embt_granule(g + 2)
    pp = proto_for_state((4 * n_gran) % 2)
    emit_out(4 * n_gran - 1, pp)
    ctx.close()


def _segment_ids(ids_dir):
    """ids_dir: (32, 512) direction-ordered ids -> per-segment (32, TLOC)."""
    segs = []
    for s in range(NSEG):
        lo = SEG * s - (WARM if s > 0 else 0)
        hi = lo + TLOC
        if hi <= T:
            seg = ids_dir[:, lo:hi]
        else:
            pad = np.repeat(ids_dir[:, -1:], hi - T, axis=1)
            seg = np.concatenate([ids_dir[:, lo:], pad], axis=1)
        segs.append(np.ascontiguousarray(seg))
    return segs


def _prep_inputs(input_ids, embed_table, w_ih_f, w_hh_f, b_ih_f, b_hh_f,
                 w_ih_b, w_hh_b, b_ih_b, b_hh_b, prototypes, n_gran=NG):
    import ml_dtypes
    bf16 = ml_dtypes.bfloat16
    ids = np.asarray(input_ids).astype(np.int32)
    emb = np.ascontiguousarray(np.asarray(embed_table, np.float32))
    prot = np.asarray(prototypes, np.float32)
    per_dir = {}
    for d, (wi, wh, bi, bh) in enumerate([
            (w_ih_f, w_hh_f, b_ih_f, b_hh_f),
            (w_ih_b, w_hh_b, b_ih_b, b_hh_b)]):
        wiA, wiB = _arrange_wih(np.asarray(wi, np.float32))
        whA, whB = _arrange_whh(np.asarray(wh, np.float32))
        bA, bB = _arrange_b(np.asarray(bi, np.float32)
                            + np.asarray(bh, np.float32))
        bA_hi = bA.astype(bf16)
        bA_lo = (bA - bA_hi.astype(np.float32)).astype(bf16)
        bB_hi = bB.astype(bf16)
        bB_lo = (bB - bB_hi.astype(np.float32)).astype(bf16)
        per_dir[d] = dict(
            wiA=np.ascontiguousarray(wiA).astype(bf16),
            wiB=np.ascontiguousarray(wiB).astype(bf16),
            whA=np.ascontiguousarray(whA).astype(bf16),
            whB=np.ascontiguousarray(whB).astype(bf16),
            bbA=np.ascontiguousarray(np.stack([bA_hi, bA_lo])),
            bbB=np.ascontiguousarray(np.stack([bB_hi, bB_lo])),
            pt=np.ascontiguousarray(
                _arrange_pt(prot[:, 512 * d:512 * (d + 1)])).astype(bf16),
        )
    in_maps = []
    for core in range(8):
        d, s = core // 4, core % 4
        ids_dir = ids if d == 0 else ids[:, ::-1]
        seg = _segment_ids(ids_dir)[s]
        pd = per_dir[d]
        in_maps.append(dict(
            emb=emb, idx=_arrange_idx(seg),
            wiA=pd["wiA"], wiB=pd["wiB"], whA=pd["whA"], whB=pd["whB"],
            bbA=pd["bbA"], bbB=pd["bbB"], pt=pd["pt"],
        ))
    return in_maps


def _combine(results, prototypes, n_gran=NG):
    p2 = (np.asarray(prototypes, np.float32) ** 2).sum(-1)  # (128,)
    out = np.zeros((32, T, 128), np.float32)
    bidx = np.arange(32)
    for core in range(8):
        d, s = core // 4, core % 4
        raw = results[core]["xp"].reshape(2, 32, TLOC, OUTW)
        blocks = raw[0] + raw[1]
        xp = blocks[:, :, 0:128]                       # (32, TLOC, 128)
        x2 = blocks[bidx, :, 128 + bidx]               # (32, TLOC)
        if s == 0:
            lo_l, hi_l, lo_t = 0, min(TLOC, SEG), 0
        else:
            lo_l = WARM
            lo_t = SEG * s
            hi_l = min(TLOC, WARM + min(SEG, T - lo_t))
        xp_r = xp[:, lo_l:hi_l]
        x2_r = x2[:, lo_l:hi_l]
        tdir = np.arange(lo_t, lo_t + hi_l - lo_l)
        tglob = tdir if d == 0 else T - 1 - tdir
        out[:, tglob, :] += 2.0 * xp_r - x2_r[:, :, None]
    out -= p2[None, None, :]
    return out


_NC_CACHE = {}


def kernel(input_ids, embed_table, w_ih_f, w_hh_f, b_ih_f, b_hh_f,
           w_ih_b, w_hh_b, b_ih_b, b_hh_b, prototypes):
    n_gran = NG
    if n_gran not in _NC_CACHE:
        _NC_CACHE[n_gran] = build_program(n_gran)
    nc = _NC_CACHE[n_gran]
    in_maps = _prep_inputs(input_ids, embed_table, w_ih_f, w_hh_f, b_ih_f,
                           b_hh_f, w_ih_b, w_hh_b, b_ih_b, b_hh_b, prototypes,
                           n_gran)
    res = run_bass_kernel_spmd(nc, in_maps, list(range(8)))
    return _combine(res.results, prototypes, n_gran)


if __name__ == "__main__":
    import time
    t0 = time.time()
    ng = int(sys.argv[1]) if len(sys.argv) > 1 else NG
    nc = build_program(ng)
    print(f"built n_gran={ng} in {time.time()-t0:.1f}s")


# revision 34
# speedup vs baseline: 1.3109x; 1.1168x over previous
"""BiLSTM + prototype-distance kernel for 8 trn2 NeuronCores.

v5 sharding: 8 cores = 2 directions x 4 SEQUENCE SEGMENTS, each core
carries the FULL batch of 32 rows. The LSTM forget gate contracts state
by ~0.5x/step, so a segment started from zero state converges to the
true state in ~32 steps; we run W=48 warmup steps (state error ~2e-7)
and discard their outputs. Per-core steps: 176 instead of 512.

v5 structure (per step, all batch-32):
- x@Wih for step t+1 runs during step t's activation chain, writing
  DIRECTLY into the (ping-ponged) G PSUM banks: two exact-bias matmuls
  (bf16 hi+lo) open the bank, 16 accumulating MMs add the embedding
  contribution. The recurrent h@Whh matmuls for step t+1 then
  accumulate into the same banks - no injection matmul, no xg ring.
- G split across PSUM banks A=[f,i,g] / B=[o]: sigmoid(f,i,g) on the
  chain, sigmoid(o) off it.
- Cell update v/u/c on DVE, tanh on ACT, h=o*tanh(c) bf16, hT2 via DVE
  32x32 stream transpose; Whh/protos host-permuted to the hT2 block
  convention (chunk k <-> hT2[:, 32k:32k+32]).
- ||h||^2 via 4 Gram matmuls on the proto PSUM tile (host reads diag).
- Embeds: GPSIMD indirect-gather (fp32) -> ACT bf16 cast -> DMA-xbar
  transposes (off the PE) -> embT.
"""

import sys
import numpy as np

sys.path.insert(0, "/opt/trn_rl_repo")

import concourse.bass as bass  # noqa: E402
import concourse.tile as tile  # noqa: E402
import concourse.mybir as mybir  # noqa: E402
from concourse import bacc  # noqa: E402
from concourse.bass_utils import run_bass_kernel_spmd  # noqa: E402

F32 = mybir.dt.float32
BF16 = mybir.dt.bfloat16
I32 = mybir.dt.int32

V, E, HD, P = 50000, 512, 1024, 128
H2 = HD // 2          # 512 per-direction hidden
B, T = 32, 512
NSEG = 4              # sequence segments per direction
WARM = 48             # warmup steps (state converges in ~32)
SEG = T // NSEG       # 128 real steps per segment
TLOC = SEG + WARM     # 176 steps per core
NG = TLOC // 4        # 44 granules
# gate order in G columns: f, i, g | o  (pytorch rows are i,f,g,o)
SRC = [1, 0, 2, 3]
OUTW = 160            # 128 proto cols + 32 gram cols per step


def _arrange_whh(w):
    """w: (2048, 512) -> A: (4, 128, 4*384), B: (4, 128, 4*128) in the
    hT2 convention: chunk k partition p <-> hidden 128*(p//32)+32k+(p%32)."""
    arrA = np.empty((4, 128, 4 * 384), np.float32)
    arrB = np.empty((4, 128, 4 * 128), np.float32)
    hi = np.arange(128)
    for k in range(4):
        hin = 128 * (hi // 32) + 32 * k + (hi % 32)
        for gam in range(4):
            scale = 2.0 if gam == 2 else 1.0
            blk = w[512 * SRC[gam]:512 * (SRC[gam] + 1), :][:, hin]  # (512,128)
            for c in range(4):
                sub = scale * blk[128 * c:128 * (c + 1), :].T        # (128,128)
                if gam < 3:
                    arrA[k, :, 384 * c + 128 * gam:384 * c + 128 * (gam + 1)] = sub
                else:
                    arrB[k, :, 128 * c:128 * (c + 1)] = sub
    return arrA, arrB


def _arrange_wih(w):
    """w: (2048, 512) -> A: (4, 128, 4*384), B: (4, 128, 4*128):
    contraction chunk k = embedding dims [128k, 128k+128) (plain order);
    gate column order f, i, g (A) | o (B), tanh gate doubled."""
    arrA = np.empty((4, 128, 4 * 384), np.float32)
    arrB = np.empty((4, 128, 4 * 128), np.float32)
    for k in range(4):
        for gam in range(4):
            scale = 2.0 if gam == 2 else 1.0
            blk = w[512 * SRC[gam]:512 * (SRC[gam] + 1),
                    128 * k:128 * (k + 1)]      # (512, 128)
            for c in range(4):
                sub = scale * blk[128 * c:128 * (c + 1), :].T
                if gam < 3:
                    arrA[k, :, 384 * c + 128 * gam:384 * c + 128 * (gam + 1)] = sub
                else:
                    arrB[k, :, 128 * c:128 * (c + 1)] = sub
    return arrA, arrB


def _arrange_b(b_total):
    """-> A (128, 384), B (128, 128) f32 in G layout."""
    bbA = np.zeros((128, 384), np.float32)
    bbB = np.zeros((128, 128), np.float32)
    for c in range(4):
        for gam in range(4):
            scale = 2.0 if gam == 2 else 1.0
            seg = scale * b_total[512 * SRC[gam] + 128 * c:
                                  512 * SRC[gam] + 128 * (c + 1)]
            if gam < 3:
                bbA[32 * c:32 * (c + 1), 128 * gam:128 * (gam + 1)] = seg[None, :]
            else:
                bbB[32 * c:32 * (c + 1), :] = seg[None, :]
    return bbA, bbB


def _arrange_pt(prot_half):
    """prot_half: (128, 512) -> (4, 128, 128) in the hT2 convention."""
    hi = np.arange(128)
    cc, jp = hi // 32, hi % 32
    arr = np.empty((4, 128, 128), np.float32)
    for k in range(4):
        hin = 128 * cc + 32 * k + jp
        arr[k] = prot_half[:, hin].T
    return arr


def _arrange_idx(ids_seg):
    """ids_seg: (32, TLOC) -> (128, NG) int32: [32*tt + b, g] = ids[b, 4g+tt]."""
    idx = np.zeros((128, NG), np.int32)
    for g in range(NG):
        for tt in range(4):
            idx[32 * tt:32 * (tt + 1), g] = ids_seg[:, 4 * g + tt]
    return idx


def build_program(n_gran=NG):
    nc = bacc.Bacc("TRN2", target_bir_lowering=False, debug=False)

    emb = nc.dram_tensor("emb", [V, E], F32, kind="ExternalInput").ap()
    idx_d = nc.dram_tensor("idx", [128, n_gran], I32, kind="ExternalInput").ap()
    wiA_d = nc.dram_tensor("wiA", [4, 128, 4 * 384], BF16, kind="ExternalInput").ap()
    wiB_d = nc.dram_tensor("wiB", [4, 128, 4 * 128], BF16, kind="ExternalInput").ap()
    whA_d = nc.dram_tensor("whA", [4, 128, 4 * 384], BF16, kind="ExternalInput").ap()
    whB_d = nc.dram_tensor("whB", [4, 128, 4 * 128], BF16, kind="ExternalInput").ap()
    bbA_d = nc.dram_tensor("bbA", [2, 128, 384], BF16, kind="ExternalInput").ap()
    bbB_d = nc.dram_tensor("bbB", [2, 128, 128], BF16, kind="ExternalInput").ap()
    pt_d = nc.dram_tensor("pt", [4, 128, 128], BF16, kind="ExternalInput").ap()

    Tloc = 4 * n_gran
    xp_d = nc.dram_tensor("xp", [64, Tloc * OUTW], F32, kind="ExternalOutput").ap()

    with tile.TileContext(nc) as tc:
        _body(tc, n_gran, emb, idx_d, wiA_d, wiB_d, whA_d, whB_d, bbA_d,
              bbB_d, pt_d, xp_d)

    nc.compile()
    return nc


def _body(tc, n_gran, emb, idx_d, wiA_d, wiB_d, whA_d, whB_d, bbA_d, bbB_d,
          pt_d, xp_d):
    nc = tc.nc
    from contextlib import ExitStack
    ctx = ExitStack()
    const = ctx.enter_context(tc.tile_pool(name="const", bufs=1))
    state = ctx.enter_context(tc.tile_pool(name="state", bufs=1))
    work = ctx.enter_context(tc.tile_pool(name="work", bufs=2))
    psum_a = ctx.enter_context(tc.tile_pool(name="psa", bufs=2, space="PSUM"))
    psum_b = ctx.enter_context(tc.tile_pool(name="psb", bufs=2, space="PSUM"))
    psum_p = ctx.enter_context(tc.tile_pool(name="psp", bufs=2, space="PSUM"))
    psum_t = ctx.enter_context(tc.tile_pool(name="pst", bufs=1, space="PSUM"))

    # ---- resident tensors -------------------------------------------------
    wiA = const.tile([128, 4 * 4 * 384], BF16)
    wiB = const.tile([128, 4 * 4 * 128], BF16)
    whA = const.tile([128, 4 * 4 * 384], BF16)
    whB = const.tile([128, 4 * 4 * 128], BF16)
    bbA = const.tile([128, 2 * 384], BF16)      # hi | lo
    bbB = const.tile([128, 2 * 128], BF16)
    pt = const.tile([128, 4 * 128], BF16)
    idx = const.tile([128, n_gran], I32)

    for k in range(4):
        nc.sync.dma_start(wiA[:, 1536 * k:1536 * (k + 1)], wiA_d[k])
        nc.sync.dma_start(wiB[:, 512 * k:512 * (k + 1)], wiB_d[k])
        nc.sync.dma_start(whA[:, 1536 * k:1536 * (k + 1)], whA_d[k])
        nc.sync.dma_start(whB[:, 512 * k:512 * (k + 1)], whB_d[k])
        nc.sync.dma_start(pt[:, 128 * k:128 * (k + 1)], pt_d[k])
    for h in range(2):
        nc.sync.dma_start(bbA[:, 384 * h:384 * (h + 1)], bbA_d[h])
        nc.sync.dma_start(bbB[:, 128 * h:128 * (h + 1)], bbB_d[h])
    nc.sync.dma_start(idx[:], idx_d[:])

    # state
    c_st = state.tile([128, 128], F32)
    hT2 = state.tile([128, 2 * 128], BF16)          # ping-pong on t%2
    emb_ring = state.tile([128, 4 * 512], F32)      # slot = g%4 (gather dst)
    embb_ring = state.tile([128, 4 * 512], BF16)    # slot = g%4 (bf16 cast)
    embT = state.tile([128, 4 * 512], BF16)         # slot = g%4; [4k x (tt,b)]
    out_ring = state.tile([96, 16 * OUTW], F32)     # rows 32:96; host adds

    nc.gpsimd.memset(c_st[:], 0.0)
    nc.gpsimd.memset(hT2[:], 0.0)
    nc.gpsimd.memset(emb_ring[:], 0.0)
    nc.gpsimd.memset(embb_ring[:], 0.0)
    nc.gpsimd.memset(embT[:], 0.0)
    nc.gpsimd.memset(out_ring[:], 0.0)

    def gather(g):
        s = 512 * (g % 4)
        nc.gpsimd.indirect_dma_start(
            out=emb_ring[:, s:s + 512],
            out_offset=None,
            in_=emb[:],
            in_offset=bass.IndirectOffsetOnAxis(ap=idx[:, g:g + 1], axis=0),
        )

    def embb_cast(g):
        s = 512 * (g % 4)
        nc.scalar.copy(embb_ring[:, s:s + 512], emb_ring[:, s:s + 512])

    def embt_chunk(g, k):
        """one PE transpose + ACT copy -> embT slot g%4, chunk k."""
        s2 = 512 * (g % 4)
        tp = psum_t.tile([128, 128], BF16, tag="tp")
        nc.tensor.matmul(
            tp[:], lhsT=embb_ring[:, s2 + 128 * k:s2 + 128 * (k + 1)],
            rhs=identb[:], is_transpose=True, start=True, stop=True)
        nc.scalar.copy(embT[:, s2 + 128 * k:s2 + 128 * (k + 1)], tp[:])

    g_tiles = {}

    def phase1_step(t):
        """xg for step t -> fresh GA/GB psum tiles (bias + x@Wih)."""
        g, tt = t // 4, t % 4
        s2 = 512 * (g % 4)
        GA = psum_a.tile([128, 384], F32, tag="ga")
        GB = psum_b.tile([128, 128], F32, tag="gb")
        g_tiles[t] = (GA, GB)
        # bias inject (bf16-rounded; residual ~6e-4 on outputs)
        nc.tensor.matmul(GA[:], lhsT=identb[:], rhs=bbA[:, 0:384],
                         start=True, stop=False)
        nc.tensor.matmul(GB[:], lhsT=identb[:], rhs=bbB[:, 0:128],
                         start=True, stop=False)
        for k in range(4):
            et = embT[:, s2 + 128 * k + 32 * tt:s2 + 128 * k + 32 * (tt + 1)]
            for c in range(4):
                nc.tensor.matmul(
                    GA[32 * c:32 * c + 32, :],
                    lhsT=et,
                    rhs=wiA[:, 1536 * k + 384 * c:1536 * k + 384 * (c + 1)],
                    start=False, stop=False,
                    tile_position=(0, 32 * c))
            for c in range(4):
                nc.tensor.matmul(
                    GB[32 * c:32 * c + 32, :],
                    lhsT=et,
                    rhs=wiB[:, 512 * k + 128 * c:512 * k + 128 * (c + 1)],
                    start=False, stop=False,
                    tile_position=(0, 32 * c))

    def step_mms(t):
        """h@Whh accumulating into the phase1-opened banks."""
        GA, GB = g_tiles.pop(t)
        cur = hT2[:, 128 * (t % 2):128 * (t % 2) + 128]
        for k in range(4):
            for c in range(4):
                nc.tensor.matmul(
                    GA[32 * c:32 * c + 32, :],
                    lhsT=cur[:, 32 * k:32 * k + 32],
                    rhs=whA[:, 1536 * k + 384 * c:1536 * k + 384 * (c + 1)],
                    start=False, stop=(k == 3),
                    tile_position=(0, 32 * c))
        for k in range(4):
            for c in range(4):
                nc.tensor.matmul(
                    GB[32 * c:32 * c + 32, :],
                    lhsT=cur[:, 32 * k:32 * k + 32],
                    rhs=whB[:, 512 * k + 128 * c:512 * k + 128 * (c + 1)],
                    start=False, stop=(k == 3),
                    tile_position=(0, 32 * c))
        return GA, GB

    def chain(t, GA, GB):
        nxt = hT2[:, 128 * ((t + 1) % 2):128 * ((t + 1) % 2) + 128]
        gh = work.tile([128, 384], F32, tag="gh")
        gho = work.tile([128, 128], F32, tag="gho")
        nc.scalar.activation(gh[:], GA[:], mybir.ActivationFunctionType.Sigmoid)
        nc.scalar.activation(gho[:], GB[:], mybir.ActivationFunctionType.Sigmoid)
        u = work.tile([128, 128], F32, tag="u")
        v = work.tile([128, 128], F32, tag="v")
        nc.vector.tensor_tensor(out=v[:], in0=gh[:, 0:128], in1=c_st[:],
                                op=mybir.AluOpType.mult)
        nc.vector.scalar_tensor_tensor(
            out=u[:], in0=gh[:, 256:384], scalar=0.5, in1=gh[:, 128:256],
            op0=mybir.AluOpType.subtract, op1=mybir.AluOpType.mult)
        nc.vector.scalar_tensor_tensor(
            out=c_st[:], in0=u[:], scalar=2.0, in1=v[:],
            op0=mybir.AluOpType.mult, op1=mybir.AluOpType.add)
        tc_t = work.tile([128, 128], F32, tag="tc")
        nc.scalar.activation(tc_t[:], c_st[:], mybir.ActivationFunctionType.Tanh)
        h_sb = work.tile([128, 128], BF16, tag="h")
        nc.vector.tensor_tensor(out=h_sb[:], in0=gho[:], in1=tc_t[:],
                                op=mybir.AluOpType.mult)
        nc.vector.transpose(nxt, h_sb[:])

    def proto_for_state(buf):
        """proto+gram as two half-sums on strips 1 and 2 (keeps strip 0
        free for the gate matmuls); emit_out adds the halves."""
        cur = hT2[:, 128 * buf:128 * buf + 128]
        pp = psum_p.tile([128, OUTW], F32)
        for h in range(2):
            sl = slice(32 + 32 * h, 64 + 32 * h)
            for kk in range(2):
                k = 2 * h + kk
                nc.tensor.matmul(pp[sl, 0:128],
                                 lhsT=cur[:, 32 * k:32 * k + 32],
                                 rhs=pt[:, 128 * k:128 * (k + 1)],
                                 start=(kk == 0), stop=False,
                                 tile_position=(0, 32 + 32 * h))
                nc.tensor.matmul(pp[sl, 128:160],
                                 lhsT=cur[:, 32 * k:32 * k + 32],
                                 rhs=cur[:, 32 * k:32 * k + 32],
                                 start=False, stop=(kk == 1),
                                 tile_position=(0, 32 + 32 * h))
        return pp

    def emit_out(tprev, pp):
        col = OUTW * (tprev % 16)
        nc.vector.tensor_copy(out_ring[32:64, col:col + OUTW], pp[32:64, :])
        nc.vector.tensor_copy(out_ring[64:96, col:col + OUTW], pp[64:96, :])
        if tprev % 16 == 15:
            blk = (tprev - 15) * OUTW
            nc.sync.dma_start(xp_d[:, blk:blk + 16 * OUTW], out_ring[32:96, :])

    # identity for the bias matmuls (declared late so make_identity's
    # gpsimd ops sit after the big memsets)
    identb = const.tile([128, 128], BF16)
    from concourse.masks import make_identity
    make_identity(nc, identb[:])

    # ---- main loop --------------------------------------------------------
    for g in range(3):
        gather(g)
    for g in range(2):
        embb_cast(g)
        for k in range(4):
            embt_chunk(g, k)
    phase1_step(0)
    for g in range(n_gran):
        if g + 3 < n_gran:
            gather(g + 3)
        if g + 2 < n_gran:
            embb_cast(g + 2)
        for tt in range(4):
            t = 4 * g + tt
            GA, GB = step_mms(t)
            pp = proto_for_state(t % 2) if t > 0 else None
            if t + 1 < 4 * n_gran:
                phase1_step(t + 1)
            if g + 2 < n_gran:
                embt_chunk(g + 2, tt)
            chain(t, GA, GB)
            if pp is not None:
                emit_out(t - 1, pp)
    pp = proto_for_state((4 * n_gran) % 2)
    emit_out(4 * n_gran - 1, pp)
    ctx.close()


def _segment_ids(ids_dir):
    """ids_dir: (32, 512) direction-ordered ids -> per-segment (32, TLOC)."""
    segs = []
    for s in range(NSEG):
        lo = SEG * s - (WARM if s > 0 else 0)
        hi = lo + TLOC
        if hi <= T:
            seg = ids_dir[:, lo:hi]
        else:
            pad = np.repeat(ids_dir[:, -1:], hi - T, axis=1)
            seg = np.concatenate([ids_dir[:, lo:], pad], axis=1)
        segs.append(np.ascontiguousarray(seg))
    return segs


def _prep_inputs(input_ids, embed_table, w_ih_f, w_hh_f, b_ih_f, b_hh_f,
                 w_ih_b, w_hh_b, b_ih_b, b_hh_b, prototypes, n_gran=NG):
    import ml_dtypes
    bf16 = ml_dtypes.bfloat16
    ids = np.asarray(input_ids).astype(np.int32)
    emb = np.ascontiguousarray(np.asarray(embed_table, np.float32))
    prot = np.asarray(prototypes, np.float32)
    per_dir = {}
    for d, (wi, wh, bi, bh) in enumerate([
            (w_ih_f, w_hh_f, b_ih_f, b_hh_f),
            (w_ih_b, w_hh_b, b_ih_b, b_hh_b)]):
        wiA, wiB = _arrange_wih(np.asarray(wi, np.float32))
        whA, whB = _arrange_whh(np.asarray(wh, np.float32))
        bA, bB = _arrange_b(np.asarray(bi, np.float32)
                            + np.asarray(bh, np.float32))
        bA_hi = bA.astype(bf16)
        bA_lo = (bA - bA_hi.astype(np.float32)).astype(bf16)
        bB_hi = bB.astype(bf16)
        bB_lo = (bB - bB_hi.astype(np.float32)).astype(bf16)
        per_dir[d] = dict(
            wiA=np.ascontiguousarray(wiA).astype(bf16),
            wiB=np.ascontiguousarray(wiB).astype(bf16),
            whA=np.ascontiguousarray(whA).astype(bf16),
            whB=np.ascontiguousarray(whB).astype(bf16),
            bbA=np.ascontiguousarray(np.stack([bA_hi, bA_lo])),
            bbB=np.ascontiguousarray(np.stack([bB_hi, bB_lo])),
            pt=np.ascontiguousarray(
                _arrange_pt(prot[:, 512 * d:512 * (d + 1)])).astype(bf16),
        )
    in_maps = []
    for core in range(8):
        d, s = core // 4, core % 4
        ids_dir = ids if d == 0 else ids[:, ::-1]
        seg = _segment_ids(ids_dir)[s]
        pd = per_dir[d]
        in_maps.append(dict(
            emb=emb, idx=_arrange_idx(seg),
            wiA=pd["wiA"], wiB=pd["wiB"], whA=pd["whA"], whB=pd["whB"],
            bbA=pd["bbA"], bbB=pd["bbB"], pt=pd["pt"],
        ))
    return in_maps


def _combine(results, prototypes, n_gran=NG):
    p2 = (np.asarray(prototypes, np.float32) ** 2).sum(-1)  # (128,)
    out = np.zeros((32, T, 128), np.float32)
    bidx = np.arange(32)
    for core in range(8):
        d, s = core // 4, core % 4
        raw = results[core]["xp"].reshape(2, 32, TLOC, OUTW)
        blocks = raw[0] + raw[1]
        xp = blocks[:, :, 0:128]                       # (32, TLOC, 128)
        x2 = blocks[bidx, :, 128 + bidx]               # (32, TLOC)
        if s == 0:
            lo_l, hi_l, lo_t = 0, min(TLOC, SEG), 0
        else:
            lo_l = WARM
            lo_t = SEG * s
            hi_l = min(TLOC, WARM + min(SEG, T - lo_t))
        xp_r = xp[:, lo_l:hi_l]
        x2_r = x2[:, lo_l:hi_l]
        tdir = np.arange(lo_t, lo_t + hi_l - lo_l)
        tglob = tdir if d == 0 else T - 1 - tdir
        out[:, tglob, :] += 2.0 * xp_r - x2_r[:, :, None]
    out -= p2[None, None, :]
    return out


_NC_CACHE = {}


def kernel(input_ids, embed_table, w_ih_f, w_hh_f, b_ih_f, b_hh_f,
           w_ih_b, w_hh_b, b_ih_b, b_hh_b, prototypes):
    n_gran = NG
    if n_gran not in _NC_CACHE:
        _NC_CACHE[n_gran] = build_program(n_gran)
    nc = _NC_CACHE[n_gran]
    in_maps = _prep_inputs(input_ids, embed_table, w_ih_f, w_hh_f, b_ih_f,
                           b_hh_f, w_ih_b, w_hh_b, b_ih_b, b_hh_b, prototypes,
                           n_gran)
    res = run_bass_kernel_spmd(nc, in_maps, list(range(8)))
    return _combine(res.results, prototypes, n_gran)


if __name__ == "__main__":
    import time
    t0 = time.time()
    ng = int(sys.argv[1]) if len(sys.argv) > 1 else NG
    nc = build_program(ng)
    print(f"built n_gran={ng} in {time.time()-t0:.1f}s")
